# revision 3
# baseline (speedup 1.0000x reference)
"""MetaLEAP edge scorer v5: HBM node-major table + SWDGE dma_gather.

Math (layer li): b0 = psi_b + delta_w[li] + u[li]
  tabN[n, j] = sum_c W[c, j] * x[n, c]     (j<10; 0:5 row-part, 5:10 col-part)
  s[e] = sum_{j<5} SF'[e,j] * (tabN[r_e, j] + tabN[c_e, 5+j])
  y[e, h] = gamma[h] * leaky_relu(s[e])

Device plan per core (edges sharded, EC = E/8):
  Phase A (49 blocks of 2048 nodes): xt block (host bf16) -> 16 flipped PE
    matmuls (lhsT = x-tile [64, 128n], rhs = W [64, 16]) -> psum [128, 256]
    -> DVE copy -> DMA to tabN[n, 0:16] f32, 256B node stride (dma_gather
    stride requirement).
  Phase B: edges bucketed by (row-chunk, col-chunk), 4 chunks < 32768 nodes
    (int16 idx). Per bucket: two dma_gather instrs (row: tabN[.., 0:5],
    col: tabN[.., 5:10]) with slot-aligned idx lists -> dr/dc [128, nb, 5]
    -> DVE: mm[.., 0:5] = dr*sf, mm[.., 5:10] = dc*sf, reduce(10) -> s,
    leaky, *gamma -> y bf16 [128, nb, 8] -> ydev.
  SWDGE desc-gen on Pool (2 Q7 cores per queue, 4 queues round-robin);
  drains on the 16 DMA engines overlap Pool gen and phase A.
Host does layout only: bf16 casts, bucket-sort, idx/SF slot packing,
unpermute + f32 upcast of y.
"""
import sys
if '/opt/trn_rl_repo' not in sys.path:
    sys.path.insert(0, '/opt/trn_rl_repo')

import numpy as np
import contextlib

import concourse.bacc as bacc
import concourse.mybir as mybir
from concourse.library_config import mlp as mlp_lib
from concourse.bass_utils import run_bass_kernel_spmd

N = 100000
C = 64
E = 1600000
H = 8
NEG = 0.01
NCORES = 8
NCH = 4
CB = (0, 26624, 51200, 75776, 100352)   # chunk boundaries (2048-aligned)
CW = (26624, 24576, 24576, 24576)
FBn = (0, 13, 25, 37)                   # chunk start in 2048-node blocks
NBC = (13, 12, 12, 12)                  # blocks per chunk
NP3 = CB[4]
XW = 2048
NBA = NP3 // XW                         # 49 phase-A blocks
EC = E // NCORES
ORDER = [(0, 0), (0, 1), (1, 0), (1, 1),
         (0, 2), (1, 2), (2, 0), (2, 1), (2, 2),
         (0, 3), (1, 3), (2, 3), (3, 0), (3, 1), (3, 2), (3, 3)]
NB = 16


def dma_gather_raw(gp, out_ap, in_ap, idxs_ap, num_idxs, num_valid,
                   elem_size, elem_step, queue_num=0):
    stride_bytes = elem_step * mybir.dt.size(in_ap.dtype)
    assert stride_bytes % 256 == 0 and stride_bytes // 256 < 256
    _in_ap = gp.lower_ap_dma(in_ap, for_custom_bir_dma=True)
    _idxs_ap = gp.lower_ap(idxs_ap)
    _out_ap = gp.lower_ap(out_ap)
    return gp.add_instruction(
        mybir.InstDMAGatherAnt(
            name=gp.bass.get_next_instruction_name(),
            ins=[*_in_ap, _idxs_ap,
                 gp.lower_val_access(gp.to_reg(num_valid))],
            outs=[_out_ap],
            transpose=False,
            num_idxs=num_idxs,
            elem_size=elem_size,
            stride_bytes_256=stride_bytes // 256,
            gen_mode=0,
            single_packet=True,
            queue_num=queue_num,
            sbuf_tokens_per_rank=0,
            sbuf_free_dim_per_rank=0,
            sbuf_free_dim_pad_per_rank=0,
            sbuf_byte_offset=0,
        ))


def build_program5(nbw, nrep=1, _ablate=()):
    """nbw[o]: slot words (slots/128, multiple of 8) for ordered bucket o.
    Each bucket side issues nbw/8 dma_gather instrs of 1024 idxs (SWDGE
    ring holds ~1024 descs)."""
    TOTW = int(sum(nbw))
    MAXW = int(max(nbw))
    W0 = np.concatenate([[0], np.cumsum(nbw)]).astype(int)
    nkw = [int(w) // 8 for w in nbw]              # 1024-idx instrs per side
    # per-queue cumulative instr counts: bucket o puts nkw[o] instrs on
    # queue (2o)%4 (row) and nkw[o] on (2o+1)%4 (col). DMA-completion
    # sems are in-order only within one ring, so waits are per-queue.
    def qof(o, side):
        return (side if o % 2 == 0 else 3 - side)
    qcum = np.zeros((NB + 1, 4), dtype=int)
    for o in range(NB):
        qcum[o + 1] = qcum[o]
        qcum[o + 1, qof(o, 0)] += nkw[o]
        qcum[o + 1, qof(o, 1)] += nkw[o]
    QT = qcum[NB]                                 # per-rep totals per queue
    abl = set(_ablate)

    nc = bacc.Bacc("TRN2", target_bir_lowering=False, debug=False,
                   num_devices=NCORES, num_swdge_queues=4,
                   detect_race_conditions=False)

    bf = mybir.dt.bfloat16
    f32 = mybir.dt.float32
    i16 = mybir.dt.int16
    xt_in = nc.dram_tensor("xt_in", [C, NP3], bf, kind="ExternalInput")
    wn = nc.dram_tensor("wn", [C, 16], bf, kind="ExternalInput")
    gma = nc.dram_tensor("gma", [128, H], f32, kind="ExternalInput")
    idxh = nc.dram_tensor("idxh", [2 * NB, 32, 8 * MAXW], i16,
                          kind="ExternalInput")
    sfh = nc.dram_tensor("sfh", [128, TOTW * 5], f32, kind="ExternalInput")
    ydev = nc.dram_tensor("ydev", [128, TOTW * H], bf, kind="ExternalOutput")
    tabN = nc.dram_tensor("tabN", [NP3, 64], f32)

    with contextlib.ExitStack() as ctx:
        e = ctx.enter_context
        xs = [e(nc.sbuf_tensor(f"xs{i}", [128, XW], bf)) for i in range(4)]
        wt = e(nc.sbuf_tensor("wt", [128, 16], bf))
        gm = e(nc.sbuf_tensor("gm", [128, H], f32))
        stg = [e(nc.sbuf_tensor(f"stg{i}", [128, 256], f32))
               for i in range(2)]
        idb = [e(nc.sbuf_tensor(f"idb{i}", [128, 8 * MAXW], i16))
               for i in range(8)]
        dr = [e(nc.sbuf_tensor(f"dr{i}", [128, 5 * MAXW], f32))
              for i in range(4)]
        dc = [e(nc.sbuf_tensor(f"dc{i}", [128, 5 * MAXW], f32))
              for i in range(4)]
        sfb = [e(nc.sbuf_tensor(f"sfb{i}", [128, 5 * MAXW], f32))
               for i in range(4)]
        mm = e(nc.sbuf_tensor("mm", [128, 10 * MAXW], f32))
        ss = e(nc.sbuf_tensor("ss", [128, MAXW], f32))
        ys = e(nc.sbuf_tensor("ys", [128, MAXW], f32))
        yb = [e(nc.sbuf_tensor(f"yb{i}", [128, H * MAXW], bf))
              for i in range(2)]
        psA = [e(nc.psum_tensor(f"psA{i}", [128, 256], f32))
               for i in range(2)]
        s_ini = e(nc.semaphore("s_ini"))
        s_xt = e(nc.semaphore("s_xt"))
        s_pa = e(nc.semaphore("s_pa"))
        s_tc = e(nc.semaphore("s_tc"))
        s_tw = e(nc.semaphore("s_tw"))
        s_ix = e(nc.semaphore("s_ix"))
        s_sf = e(nc.semaphore("s_sf"))
        s_gq = [e(nc.semaphore(f"s_gq{i}")) for i in range(4)]
        s_pb = e(nc.semaphore("s_pb"))
        s_yd = e(nc.semaphore("s_yd"))
        s_f = e(nc.semaphore("s_f"))
        block = e(nc.Block())

        @block.sync
        def _(sy):
            sy.dma_start(wt[0:C, :], wn[:]).then_inc(s_ini, 16)
            sy.dma_start(gm[:], gma[:]).then_inc(s_ini, 16)
            for rep in range(nrep):
                if rep > 0:
                    sy.wait_ge(s_xt, 16 * NBA * rep)
                    sy.wait_ge(s_pa, 16 * NBA * rep)
                    sy.wait_ge(s_tc, NBA * rep)
                    sy.wait_ge(s_tw, 16 * NBA * rep)
                    sy.wait_ge(s_ix, 16 * 2 * NB * rep)
                    sy.wait_ge(s_sf, 16 * NB * rep)
                    for qq in range(4):
                        sy.wait_ge(s_gq[qq], 16 * int(QT[qq]) * rep)
                    sy.wait_ge(s_pb, NB * rep)
                    sy.wait_ge(s_yd, 16 * NB * rep)
                    sy.sem_inc(s_f, 1)
                for b in range(NBA):
                    if 'phaseA' in abl:
                        sy.sem_inc(s_xt, 16)
                        if b >= 1:
                            sy.sem_inc(s_tw, 16)
                        continue
                    if rep * NBA + b >= 4:
                        sy.wait_ge(s_pa, 16 * (rep * NBA + b - 3))
                    sy.dma_start(xs[b % 4][0:C, :],
                                 xt_in[:, b * XW:(b + 1) * XW]
                                 ).then_inc(s_xt, 16)
                    if b >= 1:
                        F = b - 1
                        sy.wait_ge(s_tc, rep * NBA + F + 1)
                        sy.dma_start(
                            tabN[F * XW:(F + 1) * XW, 0:16].rearrange(
                                "(t p) j -> p t j", p=128),
                            stg[F % 2][:].rearrange("p (t j) -> p t j", j=16),
                        ).then_inc(s_tw, 16)
                if 'phaseA' in abl:
                    sy.sem_inc(s_tw, 16)
                else:
                    F = NBA - 1
                    sy.wait_ge(s_tc, rep * NBA + F + 1)
                    sy.dma_start(
                        tabN[F * XW:(F + 1) * XW, 0:16].rearrange(
                            "(t p) j -> p t j", p=128),
                        stg[F % 2][:].rearrange("p (t j) -> p t j", j=16),
                    ).then_inc(s_tw, 16)

        @block.scalar
        def _(sc):
            sc.wait_ge(s_ini, 32)
            for rep in range(nrep):
                if rep > 0:
                    sc.wait_ge(s_f, rep)
                for o in range(NB):
                    nw = int(nbw[o])
                    for side in range(2):
                        i = 2 * o + side
                        q = qof(o, side)
                        if o >= 4:
                            sc.wait_ge(s_gq[q],
                                       16 * (int(QT[q]) * rep
                                             + int(qcum[o - 3][q])))
                        sc.dma_start(
                            idb[i % 8][32 * q:32 * q + 32, :8 * nw],
                            idxh[i, :, :8 * nw]).then_inc(s_ix, 16)
                    if o >= 4:
                        sc.wait_ge(s_pb, NB * rep + o - 3)
                    sc.dma_start(
                        sfb[o % 4][:, :5 * nw],
                        sfh[:, 5 * W0[o]:5 * W0[o + 1]]).then_inc(s_sf, 16)
                    if o >= 1:
                        sc.wait_ge(s_pb, NB * rep + o)
                        pw = int(nbw[o - 1])
                        sc.dma_start(
                            ydev[:, H * W0[o - 1]:H * W0[o]],
                            yb[(o - 1) % 2][:, :H * pw]).then_inc(s_yd, 16)
                sc.wait_ge(s_pb, NB * rep + NB)
                pw = int(nbw[NB - 1])
                sc.dma_start(
                    ydev[:, H * W0[NB - 1]:H * W0[NB]],
                    yb[(NB - 1) % 2][:, :H * pw]).then_inc(s_yd, 16)

        @block.tensor
        def _(te):
            te.wait_ge(s_ini, 32)
            for rep in range(nrep):
                if rep > 0:
                    te.wait_ge(s_f, rep)
                for b in range(NBA):
                    if 'phaseA' in abl:
                        te.sem_inc(s_pa, 16)
                        continue
                    te.wait_ge(s_xt, 16 * (rep * NBA + b + 1))
                    if b >= 2:
                        te.wait_ge(s_tc, rep * NBA + b - 1)
                    for m in range(16):
                        te.matmul(psA[b % 2][:, 16 * m:16 * m + 16],
                                  xs[b % 4][0:C, 128 * m:128 * (m + 1)],
                                  wt[0:C, :],
                                  start=True, stop=True).then_inc(s_pa, 1)

        @block.vector
        def _(ve):
            ve.wait_ge(s_ini, 32)
            for rep in range(nrep):
                if rep > 0:
                    ve.wait_ge(s_f, rep)
                for b in range(NBA):
                    if 'phaseA' in abl:
                        ve.sem_inc(s_tc, 1)
                        continue
                    ve.wait_ge(s_pa, 16 * (rep * NBA + b) + 16)
                    if b >= 2:
                        ve.wait_ge(s_tw, 16 * (rep * NBA + b - 1))
                    ve.tensor_copy(stg[b % 2][:],
                                   psA[b % 2][:]).then_inc(s_tc, 1)
                for o in range(NB):
                    nw = int(nbw[o])
                    for qq in (qof(o, 0), qof(o, 1)):
                        ve.wait_ge(s_gq[qq],
                                   16 * (int(QT[qq]) * rep
                                         + int(qcum[o + 1][qq])))
                    ve.wait_ge(s_sf, 16 * (NB * rep + o + 1))
                    if 'pbcompute' in abl:
                        if o >= 2:
                            ve.wait_ge(s_yd, 16 * (NB * rep + o - 1))
                        ve.sem_inc(s_pb, 1)
                        continue
                    mv = mm[:].rearrange("p (n e) -> p n e", e=10)
                    ve.tensor_tensor(
                        out=mv[:, 0:nw, 0:5],
                        in0=dr[o % 4][:, :5 * nw].rearrange(
                            "p (n e) -> p n e", e=5),
                        in1=sfb[o % 4][:, :5 * nw].rearrange(
                            "p (n e) -> p n e", e=5),
                        op=mybir.AluOpType.mult)
                    ve.tensor_tensor(
                        out=mv[:, 0:nw, 5:10],
                        in0=dc[o % 4][:, :5 * nw].rearrange(
                            "p (n e) -> p n e", e=5),
                        in1=sfb[o % 4][:, :5 * nw].rearrange(
                            "p (n e) -> p n e", e=5),
                        op=mybir.AluOpType.mult)
                    ve.tensor_reduce(
                        out=ss[:, 0:nw],
                        in_=mv[:, 0:nw, :],
                        axis=mybir.AxisListType.X,
                        op=mybir.AluOpType.add)
                    ve.scalar_tensor_tensor(
                        out=ys[:, 0:nw], in0=ss[:, 0:nw], scalar=NEG,
                        in1=ss[:, 0:nw], op0=mybir.AluOpType.mult,
                        op1=mybir.AluOpType.max)
                    if o >= 2:
                        ve.wait_ge(s_yd, 16 * (NB * rep + o - 1))
                    ve.tensor_tensor(
                        out=yb[o % 2][:, :H * nw].rearrange(
                            "p (k h) -> p k h", h=H),
                        in0=ys[:, 0:nw].unsqueeze(2).broadcast_to(
                            [128, nw, H]),
                        in1=gm[:].unsqueeze(1).broadcast_to([128, nw, H]),
                        op=mybir.AluOpType.mult).then_inc(s_pb, 1)

        @block.gpsimd
        def _(gp):
            gp.load_library(mlp_lib)
            for rep in range(nrep):
                if rep > 0:
                    gp.wait_ge(s_f, rep)
                for op_ in range(0, NB, 2):
                    if op_ >= 4:
                        gp.wait_ge(s_pb, NB * rep + op_ - 2)
                    gp.wait_ge(s_ix, 16 * (2 * NB * rep + 2 * op_ + 4))
                    chs = set()
                    for o in (op_, op_ + 1):
                        chs.update(ORDER[o])
                    for ch in sorted(chs):
                        gp.wait_ge(s_tw,
                                   16 * (rep * NBA + FBn[ch] + NBC[ch]))
                    nk2 = max(nkw[op_], nkw[op_ + 1])
                    for ki in range(nk2):
                        for o in (op_, op_ + 1):
                            if ki >= nkw[o]:
                                continue
                            rc, cc = ORDER[o]
                            for side, (ch, o5) in enumerate(
                                    ((rc, 0), (cc, 5))):
                                i = 2 * o + side
                                if 'gather' in abl:
                                    gp.sem_inc(s_gq[qof(o, side)], 16)
                                    continue
                                dst = (dr if side == 0 else dc)[o % 4]
                                q = qof(o, side)
                                dma_gather_raw(
                                    gp,
                                    dst[:, 40 * ki:40 * ki + 40].rearrange(
                                        "p (n e) -> p n e", e=5),
                                    tabN[CB[ch]:CB[ch] + CW[ch],
                                         o5:o5 + 5],
                                    idb[i % 8][:, 64 * ki:64 * ki + 64],
                                    1024, 1024, 5, 64,
                                    queue_num=q,
                                ).then_inc(s_gq[q], 16)

    nc.compile()
    return nc


def prep_inputs5(x, edge_index, structural_features, layer_idx,
                 psi_w, psi_b, delta_w, u, gamma_h):
    import ml_dtypes
    li = int(layer_idx)
    b0 = np.asarray(psi_b + delta_w[li] + u[li], np.float32)      # [128]
    psi_w = np.asarray(psi_w, np.float32)
    Wm = np.zeros((C, 16), dtype=np.float32)
    Wm[:, 0:4] = psi_w[0:C]
    Wm[:, 4] = b0[0:C]
    Wm[:, 5:9] = psi_w[C:2 * C]
    Wm[:, 9] = b0[C:2 * C]
    wn = Wm.astype(ml_dtypes.bfloat16)
    gma = np.tile(np.asarray(gamma_h[li], np.float32)[None, :], (128, 1))
    xT = np.zeros((C, NP3), dtype=ml_dtypes.bfloat16)
    xT[:, :N] = np.asarray(x, np.float32).T.astype(ml_dtypes.bfloat16)

    row = np.asarray(edge_index[0], np.int64)
    col = np.asarray(edge_index[1], np.int64)
    sfp = np.concatenate([np.asarray(structural_features, np.float32),
                          np.ones((E, 1), np.float32)], axis=1)      # [E, 5]

    cbs = np.asarray(CB[1:4], np.int64)
    rch = np.searchsorted(cbs, row, side='right')
    cch = np.searchsorted(cbs, col, side='right')
    cbase = np.asarray(CB[:4], np.int64)
    b_of = {b: o for o, b in enumerate(ORDER)}
    obkt = np.asarray([b_of[(bb // NCH, bb % NCH)] for bb in range(16)],
                      dtype=np.int64)
    bucket = obkt[rch * NCH + cch]       # ordered-bucket id per edge
    orders, cnts = [], np.zeros((NCORES, NB), dtype=np.int64)
    for c in range(NCORES):
        sl = slice(c * EC, (c + 1) * EC)
        # sort by (bucket, row-node): monotone row idx streams give the
        # row-side gather descriptors DRAM row-buffer locality
        orders.append(np.lexsort((row[sl], bucket[sl])) + c * EC)
        cnts[c] = np.bincount(bucket[sl], minlength=NB)
    nbw = 8 * np.maximum(1, -(-cnts.max(axis=0) // 1024))     # [NB] words
    TOTW = int(nbw.sum())
    MAXW = int(nbw.max())
    W0 = np.concatenate([[0], np.cumsum(nbw)]).astype(int)

    in_maps, eid_all = [], []
    for c in range(NCORES):
        order = orders[c]
        boff = np.concatenate([[0], np.cumsum(cnts[c])])
        idxh = np.zeros((2 * NB, 32, 8 * MAXW), dtype=np.int16)
        sfh = np.zeros((128, TOTW * 5), dtype=np.float32)
        eids = np.full((TOTW, 128), -1, dtype=np.int64)
        for o in range(NB):
            rc, cc = ORDER[o]
            nw = int(nbw[o])
            slots = 128 * nw
            lo, hi = int(boff[o]), int(boff[o + 1])
            ids = order[lo:hi]
            rl = np.zeros(slots, dtype=np.int16)
            cl = np.zeros(slots, dtype=np.int16)
            rl[:hi - lo] = (row[ids] - cbase[rc]).astype(np.int16)
            cl[:hi - lo] = (col[ids] - cbase[cc]).astype(np.int16)
            for side, a in enumerate((rl, cl)):
                aw = a.reshape(8 * nw, 16).T                  # [16, 8nw]
                idxh[2 * o + side, :, :8 * nw] = np.tile(aw, (2, 1))
            sfv = np.zeros((slots, 5), dtype=np.float32)
            sfv[:hi - lo] = sfp[ids]
            # slot k = w*128 + p -> sfh[p, (W0[o]+w)*5 + j]
            sfh[:, 5 * W0[o]:5 * W0[o + 1]] = (
                sfv.reshape(nw, 128, 5).transpose(1, 0, 2)
                .reshape(128, nw * 5))
            ee = np.full(slots, -1, dtype=np.int64)
            ee[:hi - lo] = ids
            eids[W0[o]:W0[o + 1], :] = ee.reshape(nw, 128)
        in_maps.append({
            "xt_in": xT, "wn": wn, "gma": gma,
            "idxh": idxh, "sfh": sfh,
        })
        eid_all.append(eids)
    return in_maps, eid_all, tuple(int(v) for v in nbw)


def unshard5(results, eid_all):
    y = np.empty((E, H), dtype=np.float32)
    for c in range(NCORES):
        yd = np.asarray(results[c]["ydev"]).astype(np.float32)
        TOTW = eid_all[c].shape[0]
        blk = yd.reshape(128, TOTW, H).transpose(1, 0, 2).reshape(-1, H)
        ids = eid_all[c].reshape(-1)
        v = ids >= 0
        y[ids[v]] = blk[v]
    return y


_CACHE = {}


def kernel(**inputs):
    in_maps, eid_all, nbw = prep_inputs5(**inputs)
    if nbw not in _CACHE:
        _CACHE[nbw] = build_program5(nbw)
    res = run_bass_kernel_spmd(_CACHE[nbw], in_maps,
                               core_ids=list(range(NCORES)))
    return unshard5(res.results, eid_all)
